# revision 7
# baseline (speedup 1.0000x reference)
"""Trainium2 Bass kernel for nn_Detector (region-sum pooling + softmax).

The reference computes softmax(x.reshape(B, H*W) @ filt) where filt is a
fixed 0/1 mask selecting 10 disjoint 113x113 rectangular regions of the
1024x1024 image.  The dense GEMM is really a sparse pooling: out[b, k]
is the sum of x[b] over region k.  Only ~12% of x is ever needed, so we
DMA exactly the 10 regions per image instead of streaming all 512 MB.

Distribution: data-parallel over batch, 8 NeuronCores x 16 images each.

DMA strategy (measured on HW, all 8 cores active): a region row is 452 B
(113 f32), so every DMA descriptor costs ~30-46 ns on its ring no matter
the size/alignment/packing -- each ring is descriptor-rate-limited, not
byte-limited.  The three dynamic DMA rings process descriptors
CONCURRENTLY: the SWDGE ring (gpsimd) sustains ~227 GB/s on this
pattern, and the two HWDGE rings (sync = qSPDynamicHW, scalar =
qActDynamicHW) ~57 GB/s each.  So regions are spread 7 / 1.5 / 1.5
across gpsimd / sync / scalar, with region 8 split by row-octets
(sync: octets 0-3 -> partitions 0-63, scalar: octets 4-7 -> 64-127).
HWDGE descriptor generation is RTL (~0.8 us per region); SWDGE gen runs
on Q7 (~1.3 us per region) and pipelines with the drain.

Per region, one DMA loads rows r0..r0+111 for all 16 images: DRAM side
is the monotonic 3D slice x[:, r0:r0+112, c0:c0+113]; SBUF side is
[128, 14, 113] with partition = (batch, row-octet).  Both sides
enumerate elements in the same order, so no AP rearrange is needed
(SWDGE crashes on non-monotonic or 4D APs).  The 113th row goes via a
small HWDGE DMA on the sync queue in parallel.

Compute: VectorE reduces (rows-in-octet, cols) per region -> [128, 1]
partials; TensorE matmuls with 0/1 block-indicators [128, 16] contract
the 8 octets per batch -> PSUM [16, 10] (a separate indicator handles
region 8's split partition mapping); VectorE adds the remainder-row
partials; ScalarE does the numerically-stable softmax.
"""

import numpy as np

import concourse.bass as bass
import concourse.tile as tile
from concourse import bacc, mybir
from concourse.bass_utils import run_bass_kernel_spmd

# Problem geometry — fixed by the reference's _build_filter(1024, 1024).
B, H, W = 128, 1024, 1024
S = 113  # min(1024 // 9, 1024 // 7)
REGIONS = [(2, 1), (2, 4), (2, 7), (4, 1), (4, 3), (4, 5), (4, 7), (6, 1), (6, 4), (6, 7)]
K = len(REGIONS)
N_CORES = 8
BPC = B // N_CORES  # images per core
F32 = mybir.dt.float32
OCT, GR = 8, 14  # 112 of the 113 region rows = 8 octets x 14 rows

SW_KS = [0, 1, 2, 3, 4, 5, 6]  # gpsimd/SWDGE ring
K_SYNC, K_SCAL, K_SPLIT = 7, 9, 8
# vector issue order = estimated DMA arrival order
REDUCE_ORDER = [0, 1, 2, 3, K_SYNC, K_SCAL, 4, 5, K_SPLIT, 6]


def host_blk():
    # blk[p, b] = 1 iff p // 8 == b: sums the 8 octets per batch.
    return np.repeat(np.eye(BPC, dtype=np.float32), OCT, axis=0)


def host_blk2():
    # Region-8 split mapping: partitions 0-63 are (img, oct 0-3) from the
    # sync half, 64-127 are (img, oct 4-7) from the scalar half, so
    # p % 64 // 4 is the image for both halves.
    blk2 = np.zeros((128, BPC), dtype=np.float32)
    for p in range(128):
        blk2[p, (p % 64) // 4] = 1.0
    return blk2


def build_nc():
    nc = bacc.Bacc("TRN2", target_bir_lowering=False, debug=False)
    x = nc.declare_dram_parameter("x", [BPC, H, W], F32, isOutput=False)
    blk_d = nc.declare_dram_parameter("blk", [128, BPC], F32, isOutput=False)
    blk2_d = nc.declare_dram_parameter("blk2", [128, BPC], F32, isOutput=False)
    out = nc.declare_dram_parameter("out", [BPC, K], F32, isOutput=True)

    with tile.TileContext(nc) as tc:
        with (
            tc.tile_pool(name="reg", bufs=4) as rpool,
            tc.tile_pool(name="small", bufs=1) as spool,
            tc.tile_pool(name="psum", bufs=1, space=bass.MemorySpace.PSUM) as ppool,
        ):
            mts = {}
            # HWDGE region DMAs first: their descriptor gen is ~0.8 us of
            # RTL, so these rings start draining before Q7 finishes
            # generating the first SWDGE region.
            for k, eng in ((K_SYNC, nc.sync), (K_SCAL, nc.scalar)):
                rb, cb = REGIONS[k]
                r0, c0 = rb * S, cb * S
                mt = spool.tile([128, GR, S], F32, tag=f"hw{k}")
                eng.dma_start(out=mt[:], in_=x[:, r0:r0 + OCT * GR, c0:c0 + S])
                mts[k] = mt
            # Region 8 split by row-octets across the two HWDGE rings.
            rb, cb = REGIONS[K_SPLIT]
            r0s, c0s = rb * S, cb * S
            mt8 = spool.tile([128, GR, S], F32, tag="hw8")
            nc.sync.dma_start(
                out=mt8[0:64, :, :], in_=x[:, r0s:r0s + 4 * GR, c0s:c0s + S]
            )
            nc.scalar.dma_start(
                out=mt8[64:128, :, :],
                in_=x[:, r0s + 4 * GR:r0s + OCT * GR, c0s:c0s + S],
            )
            mts[K_SPLIT] = mt8

            # SWDGE ring: the remaining 7 regions.
            for k in SW_KS:
                rb, cb = REGIONS[k]
                r0, c0 = rb * S, cb * S
                mt = rpool.tile([128, GR, S], F32, tag="mt")
                nc.gpsimd.dma_start(
                    out=mt[:], in_=x[:, r0:r0 + OCT * GR, c0:c0 + S]
                )
                mts[k] = mt

            # Remainder row (the 113th) of every region: tiny sync DMAs.
            rem = spool.tile([BPC, K, S], F32)
            for k, (rb, cb) in enumerate(REGIONS):
                r0, c0 = rb * S, cb * S
                nc.sync.dma_start(
                    out=rem[:, k, :], in_=x[:, r0 + OCT * GR, c0:c0 + S]
                )

            # Block indicators (host-provided — engine memsets can only
            # start at partition 0/32/64/96).  Issued after the bulk DMAs
            # so they don't delay the sync ring's region drain.
            blk = spool.tile([128, BPC], F32)
            nc.sync.dma_start(out=blk[:], in_=blk_d[:])
            blk2 = spool.tile([128, BPC], F32)
            nc.sync.dma_start(out=blk2[:], in_=blk2_d[:])

            mpart = spool.tile([128, K], F32)
            for k in REDUCE_ORDER:
                nc.vector.reduce_sum(
                    out=mpart[:, k:k + 1], in_=mts[k][:],
                    axis=mybir.AxisListType.XY,
                )

            rpart = spool.tile([BPC, K], F32)
            nc.vector.reduce_sum(out=rpart[:], in_=rem[:], axis=mybir.AxisListType.X)

            # Contract octets per batch.  Region 8 has its own partition
            # mapping, so it gets its own indicator and matmul.
            py = ppool.tile([BPC, K], F32)
            nc.tensor.matmul(py[:, 0:8], blk[:], mpart[:, 0:8], start=True, stop=True)
            nc.tensor.matmul(py[:, 8:9], blk2[:], mpart[:, 8:9], start=True, stop=True)
            nc.tensor.matmul(py[:, 9:10], blk[:], mpart[:, 9:10], start=True, stop=True)

            ys = spool.tile([BPC, K], F32)
            nc.vector.tensor_add(ys[:], py[:], rpart[:])

            # Softmax over the 10 detectors, batches on partitions.
            m = spool.tile([BPC, 1], F32)
            nc.vector.reduce_max(m[:], ys[:], axis=mybir.AxisListType.X)
            negm = spool.tile([BPC, 1], F32)
            nc.vector.tensor_scalar_mul(negm[:], m[:], -1.0)
            e = spool.tile([BPC, K], F32)
            ssum = spool.tile([BPC, 1], F32)
            nc.scalar.activation(
                e[:], ys[:], mybir.ActivationFunctionType.Exp,
                bias=negm[:], accum_out=ssum[:],
            )
            rcp = spool.tile([BPC, 1], F32)
            nc.vector.reciprocal(rcp[:], ssum[:])
            o = spool.tile([BPC, K], F32)
            nc.scalar.mul(o[:], e[:], rcp[:])
            nc.sync.dma_start(out=out[:], in_=o[:])

    nc.compile()
    return nc


_NC = None


def get_nc():
    global _NC
    if _NC is None:
        _NC = build_nc()
    return _NC


def kernel(x, filt=None, **_unused):
    nc = get_nc()
    x = np.ascontiguousarray(np.asarray(x, dtype=np.float32))
    assert x.shape == (B, H, W), x.shape
    blk, blk2 = host_blk(), host_blk2()
    in_maps = [
        {"x": x[i * BPC:(i + 1) * BPC], "blk": blk, "blk2": blk2}
        for i in range(N_CORES)
    ]
    res = run_bass_kernel_spmd(nc, in_maps, list(range(N_CORES)))
    return np.concatenate([r["out"] for r in res.results], axis=0)
